# revision 40
# baseline (speedup 1.0000x reference)
"""Trainium2 Bass kernel for a 12-head causal attention block with RoPE.

Module: qkv = x @ w_qkv.T; rope(q), rope(k); causal softmax attention;
out @ w_proj.T + b_proj.  Shapes: x [4, 2048, 768], 12 heads, Dh=64.

Sharding (8 cores): core = 2*b + hg handles batch b and head-group hg
(6 heads) as 3 head-pairs.  Each core returns ONE projection partial
y^T [768, 2048] bf16 (all 3 pairs accumulated in PSUM); the host sums
the 2 head-group partials per batch and adds b_proj.

On-core dataflow (channel-major; bf16 operands, fp32 psum):
  - PE prewarmed with junk matmuls so the HAM clock gate opens before
    real compute; resident DMAs ordered so V/QK matmuls start during
    the loads.
  - QKV for ALL pairs computed up front (qt/kt per pair resident);
    RoPE with a parity-split head channel order (16-lane swap via
    stream_shuffle), multiplies split across DVE and GpSimd.
  - I-outer attention loop: for each 512-token i-block, the 3 head
    pairs run back-to-back.  Scores transposed (S^T = K Q^T), two
    heads row-packed (K=64 each, tile_position (0,0)/(64,0)) into one
    2-bank psum tile; one ACT Exp covers the pair.  Causal-block skip
    with exact diagonal trim (c0 = 128*t, t = 0..3); 0/1 triangle
    multiply on diagonal blocks only.  PV uses lhsT = [ones|V_h] so
    psum rows 0:63 carry the softmax denominator; one wide
    reciprocal + two multiplies normalize.
  - Leftover V/QK blocks and the previous i-block's projection are
    drained as PE filler inside the ACT-paced attention loop.
  - Projection per i-block contracts all 3 pairs into one psum tile
    (K=3x128), single bf16 copy, single DMA per 128-row block.
"""

import sys

sys.path.insert(0, "/opt/trn_rl_repo")

import numpy as np
import ml_dtypes

BF = ml_dtypes.bfloat16

B, N, C, H, Dh = 4, 2048, 768, 12, 64
NCORES = 8
NPAIRS = 3  # head pairs per core
NI = 4      # 512-token i-super blocks
NJ = 16     # 128-token j blocks
SCALE = Dh ** -0.5

_compiled = None


def _perm64():
    """sbuf row p_l (0..63) -> original head-channel d (parity-split order)."""
    perm = np.empty(64, dtype=np.int64)
    for p in range(64):
        q_l, m = p // 32, p % 32
        r = q_l * 16 + (m % 16)
        perm[p] = 2 * r + (0 if m < 16 else 1)
    return perm


def _build_program():
    import concourse.bass as bass
    import concourse.mybir as mybir
    import concourse.tile as tile
    from concourse import bacc

    F32 = mybir.dt.float32
    BF16 = mybir.dt.bfloat16
    AF = mybir.ActivationFunctionType
    OP = mybir.AluOpType

    nc = bacc.Bacc(None, target_bir_lowering=False)

    xT = nc.dram_tensor("xT", [C, N], BF16, kind="ExternalInput")
    wqkT = nc.dram_tensor("wqkT", [NPAIRS, C, 256], BF16, kind="ExternalInput")
    wvT = nc.dram_tensor("wvT", [C, 384], BF16, kind="ExternalInput")
    wpT = nc.dram_tensor("wpT", [NPAIRS, 128, C], BF16, kind="ExternalInput")
    c2T = nc.dram_tensor("c2T", [128, N], BF16, kind="ExternalInput")
    s2T = nc.dram_tensor("s2T", [128, N], BF16, kind="ExternalInput")
    tri01 = nc.dram_tensor("tri01", [128, 128], BF16, kind="ExternalInput")
    ypart = nc.dram_tensor("ypart", [C, N], BF16, kind="ExternalOutput")

    swap_mask = list(range(16, 32)) + list(range(0, 16))

    with tile.TileContext(nc) as tc:
        with (
            tc.tile_pool(name="res", bufs=1) as res,
            tc.tile_pool(name="mm", bufs=2, space="PSUM") as mmps,
            tc.tile_pool(name="st", bufs=2, space="PSUM") as stps,
            tc.tile_pool(name="ot", bufs=1, space="PSUM") as otps,
            tc.tile_pool(name="pt", bufs=6) as ptpool,
            tc.tile_pool(name="tmp", bufs=6) as tmppool,
            tc.tile_pool(name="onrm", bufs=2) as onrmpool,
            tc.tile_pool(name="ys", bufs=2) as yspool,
        ):
            # ---- PE prewarm: junk matmuls open the HAM clock gate while
            # the resident DMAs stream in ----
            junk = res.tile([128, 640], BF16, tag="junk")
            nc.vector.memset(junk[:], 0.0)
            for w in range(12):
                pj = mmps.tile([128, 512], F32, tag="mm", name=f"warmmm{w}")
                nc.tensor.matmul(
                    pj[:], junk[:, 0:128], junk[:, 128:640],
                    start=True, stop=True,
                )

            # ---- resident loads (order = DMA queue order).  Split the big
            # tensors so the first attention block's inputs land early. ----
            # xt rides the SP HWDGE ring; weights/tables ride the Act ring
            # (idle at startup) so both load in parallel.
            NH = N // 2
            wv = res.tile([128, 6, 384], BF16, tag="wv")
            nc.scalar.dma_start(wv[:], wvT[:].rearrange("(ct p) f -> p ct f", p=128))
            xt = res.tile([128, 6, N], BF16, tag="xt")
            for ct in range(6):
                nc.sync.dma_start(xt[:, ct, 0:NH], xT[ct * 128:(ct + 1) * 128, 0:NH])
            wqk = res.tile([128, NPAIRS, 6, 256], BF16, tag="wqk")
            nc.scalar.dma_start(
                wqk[:, 0, :, :], wqkT[0].rearrange("(ct pp) f -> pp ct f", pp=128))
            c2 = res.tile([128, N], BF16, tag="c2")
            s2 = res.tile([128, N], BF16, tag="s2")
            tri = res.tile([128, 128], BF16, tag="tri")
            nc.scalar.dma_start(c2[:, 0:NH], c2T[:, 0:NH])
            nc.scalar.dma_start(s2[:, 0:NH], s2T[:, 0:NH])
            nc.gpsimd.dma_start(tri[:], tri01[:])
            for p in range(1, NPAIRS):
                nc.gpsimd.dma_start(
                    wqk[:, p, :, :],
                    wqkT[p].rearrange("(ct pp) f -> pp ct f", pp=128))
            for ct in range(6):
                nc.sync.dma_start(xt[:, ct, NH:N], xT[ct * 128:(ct + 1) * 128, NH:N])
            nc.scalar.dma_start(c2[:, NH:N], c2T[:, NH:N])
            nc.scalar.dma_start(s2[:, NH:N], s2T[:, NH:N])
            wpj = res.tile([128, NPAIRS, C], BF16, tag="wpj")
            nc.sync.dma_start(wpj[:], wpT[:].rearrange("a p f -> p a f"))

            # prewarm the exp table load off the critical path
            warm = res.tile([1, 8], F32, tag="warm")
            nc.vector.memset(warm[:], 0.0)
            nc.scalar.activation(warm[:], warm[:], AF.Exp, scale=1.0)

            # V layout per j-block, per pair: [ones|V_A(64) | ones|V_B(64)]
            vv = res.tile([128, NJ, 768], BF16, tag="vv")
            vvt = vv[:].tensor
            # fill only the six 64-wide ones column groups per j-block
            ones_dst = bass.AP(
                tensor=vvt, offset=0,
                ap=[[NJ * 768, 128], [768, NJ], [128, 6], [1, 64]],
            )
            nc.vector.memset(ones_dst, 1.0)

            # persistent per-pair q/k tiles
            qts = [res.tile([128, NI, 512], BF16, tag=f"qt{p}", name=f"qt{p}")
                   for p in range(NPAIRS)]
            kts = [res.tile([128, NI, 512], BF16, tag=f"kt{p}", name=f"kt{p}")
                   for p in range(NPAIRS)]

            def emit_v_block(tb):
                pv = mmps.tile([128, 384], F32, tag="mm", name=f"pv{tb}")
                for ct in range(6):
                    nc.tensor.matmul(
                        pv[:], xt[:, ct, tb * 128:(tb + 1) * 128], wv[:, ct, :],
                        start=(ct == 0), stop=(ct == 5),
                    )
                dst = bass.AP(
                    tensor=vvt, offset=tb * 768 + 64,
                    ap=[[NJ * 768, 128], [256, NPAIRS], [128, 2], [1, 64]],
                )
                src = pv[:].rearrange("p (a s d) -> p a s d", a=NPAIRS, s=2, d=64)
                with tc.high_priority(offset=30):
                    nc.vector.tensor_copy(dst, src)

            def emit_rope(p, sec, tb, pqk, fast=False):
                dest = qts[p] if sec == 0 else kts[p]
                tok = slice(tb * 512, (tb + 1) * 512)
                # rope: out = psum*C2 + shuffle(psum)*S2
                tsh = tmppool.tile([128, 512], F32, tag="tsh")
                tms = tmppool.tile([128, 512], F32, tag="tms")
                tmc = tmppool.tile([128, 512], F32, tag="tmc")
                nc.vector.stream_shuffle(tsh[:], pqk[:], swap_mask)
                nc.gpsimd.tensor_tensor(tms[:], tsh[:], s2[:, tok], OP.mult)
                nc.vector.tensor_tensor(tmc[:], pqk[:], c2[:, tok], OP.mult)
                addeng = nc.vector if fast else nc.gpsimd
                addeng.tensor_tensor(dest[:, tb, :], tmc[:], tms[:], OP.add)

            def emit_qk_block(p, sec, tb, fast=False):
                # sec 0 -> q, 1 -> k
                pqk = mmps.tile([128, 512], F32, tag="mm")
                tok = slice(tb * 512, (tb + 1) * 512)
                for ct in range(6):
                    nc.tensor.matmul(
                        pqk[:], wqk[:, p, ct, sec * 128:(sec + 1) * 128],
                        xt[:, ct, tok],
                        start=(ct == 0), stop=(ct == 5),
                    )
                emit_rope(p, sec, tb, pqk, fast=fast)

            def emit_qk_block2(p, sec, tba, tbb, fast=False):
                # two token blocks sharing each ct weight chunk (LDW reuse)
                pqa = mmps.tile([128, 512], F32, tag="mm")
                pqb = mmps.tile([128, 512], F32, tag="mm")
                toka = slice(tba * 512, (tba + 1) * 512)
                tokb = slice(tbb * 512, (tbb + 1) * 512)
                for ct in range(6):
                    w = wqk[:, p, ct, sec * 128:(sec + 1) * 128]
                    nc.tensor.matmul(
                        pqa[:], w, xt[:, ct, toka],
                        start=(ct == 0), stop=(ct == 5))
                    nc.tensor.matmul(
                        pqb[:], w, xt[:, ct, tokb],
                        start=(ct == 0), stop=(ct == 5))
                emit_rope(p, sec, tba, pqa, fast=fast)
                emit_rope(p, sec, tbb, pqb, fast=fast)

            # ---- filler queues: PE work drained inside the attention loop.
            # Phased per I-block so each ACT-paced window gets the V/QK
            # blocks the NEXT I-block needs (late blocks stay late, where
            # the attention stream has PE slack).
            fillq = {i: [] for i in range(NI)}
            cur_I = [0]

            def fills_left():
                return sum(len(fillq[i]) for i in range(cur_I[0] + 1))

            def drain_fill(n=1, reserve=0):
                for _ in range(n):
                    if fills_left() <= reserve:
                        return
                    for i in range(cur_I[0] + 1):
                        if fillq[i]:
                            fillq[i].pop(0)()
                            break

            def drain_all_fills():
                for i in range(NI):
                    while fillq[i]:
                        fillq[i].pop(0)()

            junk_n = [0]

            def emit_junk(n):
                # dependency-free warm matmuls: keep the PE busy (and the
                # HAM clock gate open) across unavoidable pipeline bubbles
                for _ in range(n):
                    junk_n[0] += 1
                    pj = mmps.tile(
                        [128, 512], F32, tag="mm", name=f"jmm{junk_n[0]}")
                    nc.tensor.matmul(
                        pj[:], junk[:, 0:128], junk[:, 128:640],
                        start=True, stop=True,
                    )

            # pre-attention minimum: V j-blocks 0..3, (q0,k0) all pairs
            for tb in range(4):
                emit_v_block(tb)
            for p in range(NPAIRS):
                emit_qk_block(p, 0, 0, fast=True)
                emit_qk_block(p, 1, 0, fast=True)
            # phase i provides what I-block i+1 consumes: QK token-block
            # i+1 (rope'd q/k) and V j-blocks 4(i+1)..4(i+1)+3
            for i in range(NI - 1):
                tb = i + 1
                for p in range(NPAIRS):
                    fillq[i].append(
                        lambda p=p, tb=tb: emit_qk_block(p, 0, tb, fast=(tb == 1)))
                    fillq[i].append(
                        lambda p=p, tb=tb: emit_qk_block(p, 1, tb, fast=(tb == 1)))
                for tb4 in range(4 * tb, 4 * tb + 4):
                    fillq[i].append(lambda tb4=tb4: emit_v_block(tb4))

            def emit_proj_block(I, outNT, ocb, alt=False):
                py = mmps.tile([128, 512], F32, tag="mm")
                for p in range(NPAIRS):
                    nc.tensor.matmul(
                        py[:], wpj[:, p, ocb * 128:(ocb + 1) * 128],
                        outNT[:, p, :],
                        start=(p == 0), stop=(p == NPAIRS - 1),
                    )
                ys = yspool.tile([128, 512], BF16, tag="ys")
                if ocb % 2 == 1:
                    nc.scalar.copy(ys[:], py[:])
                else:
                    nc.vector.tensor_copy(ys[:], py[:])
                dmaeng = nc.scalar if (alt and ocb % 2 == 1) else nc.sync
                dmaeng.dma_start(
                    ypart[ocb * 128:(ocb + 1) * 128,
                          I * 512:(I + 1) * 512],
                    ys[:],
                )

            # ---- attention: I-outer, unified (pair, jb) stream so S-blocks
            # flow across pair boundaries while PVs trail by one ----
            for I in range(NI):
                outNT = onrmpool.tile([128, NPAIRS, 512], BF16, tag="outNT")
                njb = 4 * I + 4
                oabs = {}
                pabs = {}

                def emit_S(p, jb):
                    qt, kt = qts[p], kts[p]
                    t = jb - 4 * I
                    c0 = 128 * t if t >= 0 else 0
                    cs = slice(c0, 512)
                    jb4 = jb // 4
                    jbs = slice((jb % 4) * 128, (jb % 4) * 128 + 128)
                    sAB = stps.tile([128, 1024], F32, tag="sAB")
                    nc.tensor.matmul(
                        sAB[:, cs], kt[0:64, jb4, jbs], qt[0:64, I, cs],
                        start=True, stop=True, tile_position=(0, 0),
                    )
                    nc.tensor.matmul(
                        sAB[:, 512 + c0:1024],
                        kt[64:128, jb4, jbs], qt[64:128, I, cs],
                        start=True, stop=True, tile_position=(64, 0),
                    )
                    pAB = ptpool.tile([128, 1024], BF16, tag="pAB")
                    sv = sAB[:].rearrange("p (h c) -> p h c", h=2)
                    pv_ = pAB[:].rearrange("p (h c) -> p h c", h=2)
                    with tc.high_priority(offset=40):
                        nc.scalar.activation(
                            pv_[:, :, c0:512], sv[:, :, c0:512],
                            AF.Exp, scale=SCALE)
                    if t >= 0:
                        dg = slice(c0, c0 + 128)
                        dgB = slice(512 + c0, 512 + c0 + 128)
                        with tc.high_priority():
                            nc.vector.tensor_tensor(
                                pAB[:, dg], pAB[:, dg], tri[:], OP.mult)
                            nc.vector.tensor_tensor(
                                pAB[:, dgB], pAB[:, dgB], tri[:], OP.mult)
                    pabs[(p, jb)] = pAB

                def emit_PV(p, jb):
                    t = jb - 4 * I
                    c0 = 128 * t if t >= 0 else 0
                    cs = slice(c0, 512)
                    if jb == 0:
                        oabs[p] = otps.tile(
                            [128, 1024], F32, tag="oAB", name=f"oAB{I}_{p}")
                    oAB = oabs[p]
                    pAB = pabs.pop((p, jb))
                    nc.tensor.matmul(
                        oAB[:, cs], vv[:, jb, p * 256:p * 256 + 128],
                        pAB[:, cs],
                        start=(jb == 0), stop=(jb == njb - 1),
                    )
                    nc.tensor.matmul(
                        oAB[:, 512 + c0:1024],
                        vv[:, jb, p * 256 + 128:p * 256 + 256],
                        pAB[:, 512 + c0:1024],
                        start=(jb == 0), stop=(jb == njb - 1),
                    )

                def emit_norm(p):
                    oAB = oabs[p]
                    rAB = onrmpool.tile([64, 1024], F32, tag="rAB")
                    with tc.high_priority():
                        nc.vector.reciprocal_approx_fast(rAB[:], oAB[0:64, :])
                        nc.vector.tensor_tensor(
                            outNT[0:64, p, :], oAB[64:128, 0:512],
                            rAB[:, 0:512], OP.mult)
                        nc.vector.tensor_tensor(
                            outNT[64:128, p, :], oAB[64:128, 512:1024],
                            rAB[:, 512:1024], OP.mult)

                cur_I[0] = I
                seq = [(p, jb) for p in range(NPAIRS) for jb in range(njb)]
                last = NI - 1
                prev = None
                for k, (p, jb) in enumerate(seq):
                    emit_S(p, jb)
                    if I == last:
                        # hold fills back: the leftovers cover the final
                        # exp->PV drain and the last norm window
                        drain_fill(1, reserve=2)
                    else:
                        drain_fill(2 if I == 0 else 1)
                    if prev is not None:
                        emit_PV(*prev)
                        if prev[1] == njb - 1:
                            emit_norm(prev[0])
                            # cover the norm->PV(next pair) latency when the
                            # filler queue has run dry
                            if fills_left() == 0 and p == prev[0] + 1:
                                emit_junk(3)
                    prev = (p, jb)
                drain_fill(2, reserve=2)
                emit_PV(*prev)
                emit_norm(prev[0])
                if I == last:
                    # the reserved fills land here, covering the final norm
                    drain_all_fills()
                elif fills_left() == 0:
                    emit_junk(2)

                # projection of this I block; for the last I emit directly,
                # otherwise defer as filler into the next I's attention
                if I == NI - 1:
                    for ocb in range(6):
                        emit_proj_block(I, outNT, ocb, alt=True)
                        if ocb % 2 == 1:
                            emit_junk(1)
                else:
                    for start in range(0, 6, 2):
                        def pj(I=I, outNT=outNT, start=start):
                            emit_proj_block(I, outNT, start)
                            emit_proj_block(I, outNT, start + 1)
                        fillq[I + 1].append(pj)

            drain_all_fills()

    nc.compile()
    return nc


def _host_prep(x, freqs_cos, freqs_sin, mask, w_qkv, w_proj):
    """Build per-core input maps."""
    perm = _perm64()

    r_of_p = np.empty(128, dtype=np.int64)
    sign_of_p = np.empty(128, dtype=np.float32)
    for pp in range(128):
        p_l = pp % 64
        q_l, m = p_l // 32, p_l % 32
        r_of_p[pp] = q_l * 16 + (m % 16)
        sign_of_p[pp] = -1.0 if m < 16 else 1.0
    c2T = np.ascontiguousarray(freqs_cos.T[r_of_p, :], dtype=np.float32)
    s2T = np.ascontiguousarray(
        freqs_sin.T[r_of_p, :] * sign_of_p[:, None], dtype=np.float32)

    # 0/1 lower-triangle (transposed causal) tile from the provided mask:
    # valid (j <= i) where mask[0,0,i,j] == 0 -> tri01[j, i] = 1
    m0 = mask[0, 0, :128, :128]
    tri01 = np.ascontiguousarray((m0.T == 0).astype(np.float32))

    in_maps = []
    for core in range(NCORES):
        b, hg = core // 2, core % 2
        heads = [hg * 6 + i for i in range(6)]
        xTc = np.ascontiguousarray(x[b].T)

        wqkT = np.empty((NPAIRS, C, 256), dtype=np.float32)
        wpTc = np.empty((NPAIRS, 128, C), dtype=np.float32)
        for p in range(NPAIRS):
            for hh in range(2):
                hgl = heads[2 * p + hh]
                rows_q = 0 * C + hgl * 64 + perm
                rows_k = 1 * C + hgl * 64 + perm
                wqkT[p, :, hh * 64:(hh + 1) * 64] = w_qkv[rows_q, :].T
                wqkT[p, :, 128 + hh * 64:128 + (hh + 1) * 64] = w_qkv[rows_k, :].T
                wpTc[p, hh * 64:(hh + 1) * 64, :] = \
                    w_proj[:, hgl * 64:(hgl + 1) * 64].T
        wvTc = np.empty((C, 384), dtype=np.float32)
        for i, hgl in enumerate(heads):
            rows_v = 2 * C + hgl * 64 + np.arange(64)
            wvTc[:, i * 64:(i + 1) * 64] = w_qkv[rows_v, :].T

        in_maps.append({
            "xT": xTc.astype(BF),
            "wqkT": np.ascontiguousarray(wqkT).astype(BF),
            "wvT": wvTc.astype(BF),
            "wpT": np.ascontiguousarray(wpTc).astype(BF),
            "c2T": c2T.astype(BF),
            "s2T": s2T.astype(BF),
            "tri01": tri01.astype(BF),
        })
    return in_maps


def _mask_is_causal(mask):
    m = mask[0, 0]
    if m.shape != (N, N):
        return False
    iu = np.triu_indices(N, k=1)
    il = np.tril_indices(N, k=0)
    return bool(np.all(m[il] == 0.0) and np.all(m[iu] <= -1e8))


def _numpy_reference(x, freqs_cos, freqs_sin, mask, w_qkv, w_proj, b_proj):
    """Exact fallback (never expected: setup_inputs' mask is causal)."""
    Bq, Nq, Cq = x.shape
    qkv = x @ w_qkv.T
    qkv = qkv.reshape(Bq, Nq, 3, H, Dh)
    q, k, v = qkv[:, :, 0], qkv[:, :, 1], qkv[:, :, 2]

    def rope(t):
        tr = t.reshape(Bq, Nq, H, Dh // 2, 2)
        a, b = tr[..., 0], tr[..., 1]
        c = freqs_cos[None, :, None, :]
        s = freqs_sin[None, :, None, :]
        return np.stack([a * c - b * s, a * s + b * c], axis=-1).reshape(t.shape)

    q, k = rope(q), rope(k)
    q = q.transpose(0, 2, 1, 3)
    k = k.transpose(0, 2, 1, 3)
    v = v.transpose(0, 2, 1, 3)
    att = np.einsum('bhqd,bhkd->bhqk', q, k) * SCALE + mask
    att = att - att.max(axis=-1, keepdims=True)
    att = np.exp(att)
    att = att / att.sum(axis=-1, keepdims=True)
    out = np.einsum('bhqk,bhkd->bhqd', att, v)
    out = out.transpose(0, 2, 1, 3).reshape(Bq, Nq, Cq)
    return (out @ w_proj.T + b_proj).astype(np.float32)


def _get_compiled():
    global _compiled
    if _compiled is None:
        _compiled = _build_program()
    return _compiled


def run_device(in_maps, trace=False, trace_kwargs=None):
    from concourse.bass_utils import run_bass_kernel_spmd
    nc = _get_compiled()
    kwargs = {}
    if trace:
        kwargs["trace"] = True
        if trace_kwargs:
            kwargs["trace_kwargs"] = trace_kwargs
    return run_bass_kernel_spmd(nc, in_maps, core_ids=list(range(NCORES)), **kwargs)


def _assemble(results, b_proj):
    y = np.empty((B, N, C), dtype=np.float32)
    for b in range(B):
        acc = results[2 * b]["ypart"].astype(np.float32)
        acc = acc + results[2 * b + 1]["ypart"].astype(np.float32)
        y[b] = acc.T + b_proj[None, :]
    return y


def kernel(x, freqs_cos, freqs_sin, mask, w_qkv, w_proj, b_proj):
    x = np.asarray(x, dtype=np.float32)
    freqs_cos = np.asarray(freqs_cos, dtype=np.float32)
    freqs_sin = np.asarray(freqs_sin, dtype=np.float32)
    mask = np.asarray(mask, dtype=np.float32)
    w_qkv = np.asarray(w_qkv, dtype=np.float32)
    w_proj = np.asarray(w_proj, dtype=np.float32)
    b_proj = np.asarray(b_proj, dtype=np.float32)

    if x.shape != (B, N, C) or not _mask_is_causal(mask):
        return _numpy_reference(x, freqs_cos, freqs_sin, mask, w_qkv, w_proj, b_proj)

    in_maps = _host_prep(x, freqs_cos, freqs_sin, mask, w_qkv, w_proj)
    res = run_device(in_maps)
    return _assemble(res.results, b_proj)
